# revision 2
# baseline (speedup 1.0000x reference)
"""Contrastive loss (CLIP-style BCE) on 8 Trainium2 NeuronCores.

Data-parallel over batch: each core takes a 128-row shard of img_features
(75.5 MB) plus replicated text_embeds/labels, computes its [128, 1024] slice
of the logits and a partial sum of softplus(x) - x*targets; the host sums the
8 partial scalars.

The dominant cost is streaming the img shard for the H*W pooling, so the
kernel is built around DMA efficiency:

- Layout B: partitions = batch rows, so each DMA descriptor is a contiguous
  per-partition run of ~36KB (img[b, c0:c0+32, :]). This runs ~4-5x faster
  than the transposing [c, b, hw] pattern (2.3KB descriptors) the previous
  version used; pooled also lands directly in [b, c] layout so no transpose
  is needed before the row norms.
- The pooling chunks stream on both HWDGE queues (SP + ACT) in f32 mode, or
  through the Pool SWDGE queue with an f32->f16 cast in fp16 mode (halves
  SBUF write traffic and DVE read traffic; loss-level error ~1e-6).
- The hw-reduce is split DVE/ACT (f32) or all-DVE (fp16, 2x element rate) so
  it hides completely under the DMA stream.

Runtime notes (established on this axon/fakenrt stack):
- PE is_transpose matmuls and InstTensorTensorReduce crash the exec unit;
  transposes are regular matmuls against identity.
- No Sqrt/Softplus activation tables: softplus = ln(exp(x)+1) (|x| <= 1/0.07
  so exp is safe), rsqrt = exp(-0.5*ln(x)) + one Newton step.
- Effective DMA bandwidth swings ~3x with ambient co-tenant load; the kernel
  tracks its DMA floor in both regimes.
"""

import numpy as np

import concourse.bacc as bacc
import concourse.mybir as mybir
import concourse.tile as tile
from concourse.bass_utils import run_bass_kernel_spmd
from concourse.masks import make_identity

N_CORES = 8
B, C, H, W = 1024, 256, 24, 24
HW = H * W  # 576
BS = B // N_CORES  # 128
P = 128
TEMP = 0.07
INV_TEMP = 1.0 / TEMP

F32 = mybir.dt.float32
F16 = mybir.dt.float16
ALU = mybir.AluOpType
ACT = mybir.ActivationFunctionType
AX = mybir.AxisListType

_NC_CACHE = []


def _emit_rsqrt(nc, small, ns, out_rv, tag):
    """out_rv = 1/sqrt(ns) via exp(-0.5*ln(ns)) + one Newton step."""
    y0 = small.tile([P, 1], F32, tag=f"{tag}_y0", name=f"{tag}_y0")
    nc.scalar.activation(y0, ns, ACT.Ln)
    nc.scalar.activation(y0, y0, ACT.Exp, scale=-0.5)
    t1 = small.tile([P, 1], F32, tag=f"{tag}_t1", name=f"{tag}_t1")
    nc.vector.tensor_mul(t1, y0, y0)
    nc.vector.tensor_mul(t1, t1, ns)
    nc.vector.tensor_scalar(
        out=t1, in0=t1, scalar1=-0.5, scalar2=1.5, op0=ALU.mult, op1=ALU.add
    )
    nc.vector.tensor_mul(out_rv, y0, t1)


def _emit_body(nc, pools, identity, ones, img, txt, lab_row, lab_all, out, cfg):
    consts, big, txtp, small, persist, scrp, psum_tp, psum_g = pools
    cchunk = cfg.get("cchunk", 16)
    dt16 = cfg.get("dt16", True)
    queues = cfg.get("queues", ("sync", "scalar"))
    w = cfg.get("rw", (16, 0) if dt16 else (9, 7))
    seq = ["dve_f"] * w[0] + ["act_f"] * w[1]

    # ---- txt path first: fills DVE/ACT/PE idle time under the first img
    # DMAs; its small loads lead the sync/scalar queues (~2us). ----
    txtT = [
        persist.tile([P, B], F32, tag=f"txtT{cb}", name=f"txtT{cb}") for cb in range(2)
    ]
    for tb in range(B // P):
        tt = txtp.tile([P, C], F32, tag="ttin", name="ttin")
        nc.sync.dma_start(out=tt, in_=txt[tb * P : (tb + 1) * P, :])
        tns = small.tile([P, 1], F32, tag="tns", name="tns")
        sq_scr = small.tile([P, C], F32, tag="sq_scr", name="sq_scr")
        nc.scalar.activation(sq_scr, tt, ACT.Square, accum_out=tns)
        trv = small.tile([P, 1], F32, tag="trv", name="trv")
        _emit_rsqrt(nc, small, tns, trv, "trsq")
        tn = txtp.tile([P, C], F32, tag="ttn", name="ttn")
        nc.vector.tensor_scalar_mul(tn, tt, trv)
        for cb in range(2):
            pt = psum_tp.tile([P, P], F32, tag="pt", name="pt")
            nc.tensor.matmul(
                pt, tn[:, cb * P : (cb + 1) * P], identity, start=True, stop=True
            )
            nc.scalar.copy(txtT[cb][:, tb * P : (tb + 1) * P], pt)

    # ---- targets[p, j] = (lab_row[p] == lab_all[j]) via relu(1-d^2),
    # with 1/T folded in ----
    lab_row_sb = small.tile([P, 1], F32, tag="lab_row_sb", name="lab_row_sb")
    nc.sync.dma_start(out=lab_row_sb, in_=lab_row)
    tgt = persist.tile([P, B], F32, tag="tgt", name="tgt")
    nc.scalar.dma_start(out=tgt, in_=lab_all.to_broadcast([P, B]))
    nc.vector.tensor_scalar_sub(tgt, tgt, lab_row_sb)
    nc.scalar.activation(tgt, tgt, ACT.Square)
    nc.scalar.activation(tgt, tgt, ACT.Relu, scale=-1.0, bias=1.0)
    nc.scalar.mul(tgt, tgt, INV_TEMP)

    # ---- pooling: pooled[b, c] = sum_hw img[b, c, hw]  (mean/576 cancels
    # under l2-normalization) ----
    pooled = persist.tile([P, C], F32, tag="pooled", name="pooled")
    k = 0
    for ci, c0 in enumerate(range(0, C, cchunk)):
        it = big.tile([P, cchunk, HW], F16 if dt16 else F32, tag="imgin", name="imgin")
        if dt16:
            # only SWDGE (Pool) DMA can cast f32 -> f16 in flight
            nc.gpsimd.dma_start(out=it, in_=img[:, c0 : c0 + cchunk, :])
        else:
            eng = getattr(nc, queues[ci % len(queues)])
            eng.dma_start(out=it, in_=img[:, c0 : c0 + cchunk, :])
        for j in range(cchunk):
            dst = pooled[:, c0 + j : c0 + j + 1]
            src = it[:, j, :]
            use = seq[k % len(seq)]
            k += 1
            if use == "dve_f":
                nc.vector.reduce_sum(out=dst, in_=src, axis=AX.X)
            else:
                ascr = scrp.tile([P, HW], F32, tag="ascr", name="ascr")
                nc.scalar.activation(ascr, src, ACT.Identity, accum_out=dst)

    # ---- pooled row norms (already [b, c] layout) ----
    ns = small.tile([P, 1], F32, tag="ns", name="ns")
    psq_scr = small.tile([P, C], F32, tag="psq_scr", name="psq_scr")
    nc.scalar.activation(psq_scr, pooled, ACT.Square, accum_out=ns)
    rv = small.tile([P, 1], F32, tag="rv", name="rv")
    _emit_rsqrt(nc, small, ns, rv, "prsq")
    pooled_n = persist.tile([P, C], F32, tag="pooled_n", name="pooled_n")
    nc.vector.tensor_scalar_mul(pooled_n, pooled, rv)
    pnT = [
        persist.tile([P, P], F32, tag=f"pnT{cb}", name=f"pnT{cb}") for cb in range(2)
    ]
    for cb in range(2):
        pq = psum_tp.tile([P, P], F32, tag="pt", name="pt")
        nc.tensor.matmul(
            pq, pooled_n[:, cb * P : (cb + 1) * P], identity, start=True, stop=True
        )
        nc.scalar.copy(pnT[cb][:], pq)

    # ---- gram [128, 1024] + softplus/target accumulation ----
    sp_acc = small.tile([P, 2], F32, tag="sp_acc", name="sp_acc")
    xt_acc = small.tile([P, 2], F32, tag="xt_acc", name="xt_acc")
    for nbk in range(2):
        g = psum_g.tile([P, 512], F32, tag="g", name="g")
        for cb in range(2):
            nc.tensor.matmul(
                g,
                pnT[cb][:],
                txtT[cb][:, nbk * 512 : (nbk + 1) * 512],
                start=(cb == 0),
                stop=(cb == 1),
            )
        # softplus(x) = ln(exp(x) + 1); |x| <= 1/0.07 so exp can't overflow
        e_scr = small.tile([P, 512], F32, tag="e_scr", name="e_scr")
        nc.scalar.activation(e_scr, g, ACT.Exp, scale=INV_TEMP)
        sp_scr = small.tile([P, 512], F32, tag="sp_scr", name="sp_scr")
        nc.scalar.activation(
            sp_scr, e_scr, ACT.Ln, bias=1.0, accum_out=sp_acc[:, nbk : nbk + 1]
        )
        xt_scr = small.tile([P, 512], F32, tag="xt_scr", name="xt_scr")
        nc.vector.tensor_mul(xt_scr, g, tgt[:, nbk * 512 : (nbk + 1) * 512])
        nc.vector.reduce_sum(out=xt_acc[:, nbk : nbk + 1], in_=xt_scr, axis=AX.X)

    # ---- total per partition, then 128-way reduce via matmul ----
    tot = small.tile([P, 1], F32, tag="tot", name="tot")
    nc.vector.reduce_sum(out=tot, in_=sp_acc, axis=AX.X)
    xtt = small.tile([P, 1], F32, tag="xtt", name="xtt")
    nc.vector.reduce_sum(out=xtt, in_=xt_acc, axis=AX.X)
    nc.vector.tensor_sub(tot, tot, xtt)
    ps = psum_tp.tile([1, 1], F32, tag="ps", name="ps")
    nc.tensor.matmul(ps, tot, ones, start=True, stop=True)
    res = small.tile([1, 1], F32, tag="res", name="res")
    nc.scalar.copy(res, ps)
    nc.sync.dma_start(out=out, in_=res)


def _build_nc(reps=1, **cfg):
    nc = bacc.Bacc("TRN2", target_bir_lowering=False, debug=False, num_devices=N_CORES)
    img = nc.dram_tensor("img", [BS, C, HW], F32, kind="ExternalInput").ap()
    txt = nc.dram_tensor("txt", [B, C], F32, kind="ExternalInput").ap()
    lab_row = nc.dram_tensor("lab_row", [BS, 1], F32, kind="ExternalInput").ap()
    lab_all = nc.dram_tensor("lab_all", [1, B], F32, kind="ExternalInput").ap()
    outs = [
        nc.dram_tensor(
            "partial" if r == 0 else f"partial{r}", [1, 1], F32, kind="ExternalOutput"
        ).ap()
        for r in range(reps)
    ]
    big_bufs = cfg.get("bufs", 8 if cfg.get("dt16", True) else 4)

    with tile.TileContext(nc) as tc:
        with (
            tc.tile_pool(name="consts", bufs=1) as consts,
            tc.tile_pool(name="big", bufs=big_bufs) as big,
            tc.tile_pool(name="txtp", bufs=3) as txtp,
            tc.tile_pool(name="small", bufs=2) as small,
            tc.tile_pool(name="persist", bufs=1) as persist,
            tc.tile_pool(name="scr", bufs=2) as scrp,
            tc.tile_pool(name="psum_tp", bufs=2, space="PSUM") as psum_tp,
            tc.tile_pool(name="psum_g", bufs=2, space="PSUM") as psum_g,
        ):
            identity = consts.tile([P, P], F32, tag="identity")
            make_identity(nc, identity)
            ones = consts.tile([P, 1], F32, tag="ones")
            nc.vector.memset(ones, 1.0)
            pools = (consts, big, txtp, small, persist, scrp, psum_tp, psum_g)
            for r in range(reps):
                _emit_body(
                    nc, pools, identity, ones, img, txt, lab_row, lab_all, outs[r], cfg
                )

    nc.finalize()
    return nc


def _get_nc():
    if not _NC_CACHE:
        _NC_CACHE.append(_build_nc())
    return _NC_CACHE[0]


def kernel(img_features, text_embeds, labels):
    img_features = np.ascontiguousarray(np.asarray(img_features, dtype=np.float32))
    text_embeds = np.ascontiguousarray(np.asarray(text_embeds, dtype=np.float32))
    labels_f = np.asarray(labels).astype(np.float32)  # values < 16: exact in f32

    img3 = img_features.reshape(B, C, HW)
    nc = _get_nc()
    in_maps = []
    for i in range(N_CORES):
        sl = slice(i * BS, (i + 1) * BS)
        in_maps.append(
            {
                "img": img3[sl],
                "txt": text_embeds,
                "lab_row": labels_f[sl].reshape(BS, 1),
                "lab_all": labels_f.reshape(1, B),
            }
        )
    for attempt in range(2):
        r = run_bass_kernel_spmd(nc, in_maps, core_ids=list(range(N_CORES)))
        total = sum(float(r.results[i]["partial"][0, 0]) for i in range(N_CORES))
        if np.isfinite(total):
            break
        # one retry: a cold-start transient produced a non-finite partial once
    return np.float32(total / (B * B))
